# revision 12
# baseline (speedup 1.0000x reference)
"""GRU kernel for Trainium2, 8 NeuronCores, data-parallel over batch.

Math (input dim == latent dim, shared weights between input and recurrent
projections lets everything fuse):
    u_t   = x_t + h_{t-1}
    z_t   = sigmoid(u_t @ Wz.T)
    s_t   = 1 - r_t = sigmoid(-(u_t @ Wr.T))
    v_t   = x_t + r_t*h = u_t - s_t*h_{t-1}
    htl_t = tanh(v_t @ W.T + 2b)
    h_t   = h_{t-1} + z_t*(htl_t - h_{t-1})

Device layout: everything lives as [128 partitions, NT*BSH] tiles where
partition p of column block m holds latent/feature index m*128+p and the
8 columns within a block are the per-core batch elements. Weights are the
stationary matmul operand (one [128,128] tile per (m,k)), the activations
are the moving operand ([128, 8] slices), so no transposes are needed
anywhere in the loop.

Weight dtype is configurable per matrix (bf16 or fp8 e4m3/e3m4). fp8
weights are stored pre-scaled by S (so the fp8 exponent range is used);
the 1/S compensation folds for free into the activation `scale` args:
    z = sigmoid(z_ps / Sz),  s = sigmoid(-r_ps / Sr),
    ht = tanh((c_ps + S*2b) / S)  [bias tile pre-scaled host-side].
The moving operand stays bf16 in all cases (mixed fp8xbf16 matmul).
"""

import os
import sys

import numpy as np

sys.path.insert(0, "/opt/trn_rl_repo")

import ml_dtypes  # noqa: E402

import concourse.bass as bass  # noqa: E402
import concourse.bacc as bacc  # noqa: E402
import concourse.mybir as mybir  # noqa: E402
import concourse.tile as tile  # noqa: E402
from concourse.bass import ds, ts  # noqa: E402
from concourse.bass_utils import run_bass_kernel_spmd  # noqa: E402

SEQ, BATCH, DIM = 512, 64, 1024
NCORES = 8
BSH = BATCH // NCORES  # batch per core = 8
NT = DIM // 128  # 8 latent tiles
FREE = NT * BSH  # 64 free columns
PAD = SEQ + 16  # x padded in seq for prefetch overrun

F32 = mybir.dt.float32
BF16 = mybir.dt.bfloat16
AF = mybir.ActivationFunctionType
OP = mybir.AluOpType

# per-dtype weight pre-scale: keeps fp8 exponent range busy while staying
# clear of max-normal (trn e4m3 saturates at 240, e3m4 at 15.5; |W|<=1/32)
W_SCALE = {
    mybir.dt.float32: 1.0,
    mybir.dt.bfloat16: 1.0,
    mybir.dt.float8e4: 4096.0,
    mybir.dt.float8e3: 256.0,
}
DT_BY_NAME = {
    "float32": F32,
    "bfloat16": BF16,
    "float8e4": mybir.dt.float8e4,
    "float8e3": mybir.dt.float8e3,
}
NP_BY_DT = {
    F32: np.float32,
    BF16: ml_dtypes.bfloat16,
    mybir.dt.float8e4: ml_dtypes.float8_e4m3,
    mybir.dt.float8e3: ml_dtypes.float8_e3m4,
}

# weight-dtype configs: (wz_dt, wr_dt, w_dt)
CONFIGS = {
    "float32": (F32, F32, F32),
    "bfloat16": (BF16, BF16, BF16),
    "float8e4": (mybir.dt.float8e4,) * 3,
    "float8e3": (mybir.dt.float8e3,) * 3,
    "hybrid_e3": (mybir.dt.float8e3, mybir.dt.float8e3, BF16),
    # both operands e3m4 (moving scaled by MOV_SCALE)
    "pure_e3": (mybir.dt.float8e3,) * 3,
}
# moving-operand dtype + scale per config (default bf16, scale 1)
MOV_DT = {"pure_e3": mybir.dt.float8e3}
MOV_SCALE = {"pure_e3": 2.0}


def build_nc(seq=SEQ, unroll=2, cfg="bfloat16", staggered=False, reps=1,
             hwreps=False):
    """Build the Bass program (shared by all 8 cores, SPMD)."""
    nc = bacc.Bacc()
    probe = cfg == "probe_mm"
    if probe:
        cfg = "bfloat16"
    wz_dt, wr_dt, w_dt = CONFIGS[cfg]
    sm = MOV_SCALE.get(cfg, 1.0)
    sz, sr, sw = (W_SCALE[d] * sm for d in (wz_dt, wr_dt, w_dt))
    all_f32 = cfg == "float32"
    mov_dt = F32 if all_f32 else MOV_DT.get(cfg, BF16)
    cast_rhs = not all_f32

    x_d = nc.declare_dram_parameter("x", [PAD * 128, FREE], F32, isOutput=False)
    wz_d = nc.declare_dram_parameter("wz", [128, NT * NT * 128], wz_dt, isOutput=False)
    wr_d = nc.declare_dram_parameter("wr", [128, NT * NT * 128], wr_dt, isOutput=False)
    w_d = nc.declare_dram_parameter("w", [128, NT * NT * 128], w_dt, isOutput=False)
    b_d = nc.declare_dram_parameter("bias2", [128, FREE], F32, isOutput=False)
    out_d = nc.declare_dram_parameter("out", [seq * 128, FREE], F32, isOutput=True)

    assert seq % unroll == 0 and unroll % 2 == 0

    with tile.TileContext(nc) as tc:
        with (
            tc.tile_pool(name="pers", bufs=1) as pers,
            tc.tile_pool(name="tmp", bufs=2) as tmp,
            tc.tile_pool(name="psum", bufs=2, space="PSUM") as psum,
        ):
            wz = pers.tile([128, NT * NT * 128], wz_dt, tag="wz")
            wr = pers.tile([128, NT * NT * 128], wr_dt, tag="wr")
            w = pers.tile([128, NT * NT * 128], w_dt, tag="w")
            bias2 = pers.tile([128, FREE], F32, tag="bias2")
            nc.sync.dma_start(out=wz, in_=wz_d[:])
            nc.sync.dma_start(out=wr, in_=wr_d[:])
            nc.sync.dma_start(out=w, in_=w_d[:])
            nc.sync.dma_start(out=bias2, in_=b_d[:])

            # ping-pong state slots
            h = [pers.tile([128, FREE], F32, tag=f"h{i}", name=f"h{i}") for i in range(2)]
            u = [pers.tile([128, FREE], F32, tag=f"u{i}", name=f"u{i}") for i in range(2)]
            ub = (
                [pers.tile([128, FREE], mov_dt, tag=f"ub{i}", name=f"ub{i}") for i in range(2)]
                if cast_rhs
                else u
            )
            xs = [pers.tile([128, FREE], F32, tag=f"xs{i}", name=f"xs{i}") for i in range(unroll)]

            def cast_mov(dst, src):
                if sm == 1.0:
                    nc.vector.tensor_copy(dst, src)
                else:
                    nc.scalar.mul(dst, src, sm)

            def prologue():
                nc.vector.memset(h[0], 0.0)
                # u_0 = x_0 + h_0 = x_0
                nc.sync.dma_start(out=u[0], in_=x_d[0:128, :])
                if cast_rhs:
                    cast_mov(ub[0], u[0])
                for s in range(unroll):
                    nc.sync.dma_start(
                        out=xs[s], in_=x_d[(s + 1) * 128 : (s + 2) * 128, :]
                    )

            def substep(s, off):
                """off = dram row offset (AP expr) of step t; slot parity p."""
                p, q = s % 2, (s + 1) % 2
                r_ps = psum.tile([128, FREE], F32, tag="r_ps", name="r_ps")
                z_ps = psum.tile([128, FREE], F32, tag="z_ps", name="z_ps")
                c_ps = psum.tile([128, FREE], F32, tag="c_ps", name="c_ps")

                # r gate first (its sigmoid is on the critical path to cand)
                for m in range(NT):
                    for k in range(NT):
                        nc.tensor.matmul(
                            r_ps[:, ts(m, BSH)],
                            wr[:, ds((m * NT + k) * 128, 128)],
                            ub[p][:, ts(k, BSH)],
                            start=(k == 0),
                            stop=(k == NT - 1),
                        )
                for m in range(NT):
                    for k in range(NT):
                        nc.tensor.matmul(
                            z_ps[:, ts(m, BSH)],
                            wz[:, ds((m * NT + k) * 128, 128)],
                            ub[p][:, ts(k, BSH)],
                            start=(k == 0),
                            stop=(k == NT - 1),
                        )

                # s = 1 - r = sigmoid(-r_pre); r_ps holds Sr * r_pre
                s_sb = tmp.tile([128, FREE], F32, tag="s_sb", name="s_sb")
                nc.scalar.activation(s_sb, r_ps, AF.Sigmoid, scale=-1.0 / sr)
                # v = u - s*h
                sh = tmp.tile([128, FREE], F32, tag="sh", name="sh")
                nc.vector.tensor_mul(sh, s_sb, h[p])
                v = tmp.tile([128, FREE], F32, tag="v", name="v")
                nc.vector.tensor_sub(v, u[p], sh)
                if cast_rhs:
                    vb = tmp.tile([128, FREE], mov_dt, tag="vb", name="vb")
                    cast_mov(vb, v)
                else:
                    vb = v

                for m in range(NT):
                    for k in range(NT):
                        nc.tensor.matmul(
                            c_ps[:, ts(m, BSH)],
                            w[:, ds((m * NT + k) * 128, 128)],
                            vb[:, ts(k, BSH)],
                            start=(k == 0),
                            stop=(k == NT - 1),
                        )

                # z while cand runs on PE; z_ps holds Sz * z_pre
                z_sb = tmp.tile([128, FREE], F32, tag="z_sb", name="z_sb")
                nc.scalar.activation(z_sb, z_ps, AF.Sigmoid, scale=1.0 / sz)

                # htilde = tanh((c + Sw*bias2)/Sw); bias2 tile pre-scaled by Sw
                ct = tmp.tile([128, FREE], F32, tag="ct", name="ct")
                nc.vector.tensor_add(ct, c_ps, bias2)
                ht = tmp.tile([128, FREE], F32, tag="ht", name="ht")
                nc.scalar.activation(ht, ct, AF.Tanh, scale=1.0 / sw)

                # h_new = h + z*(ht - h)
                d_ = tmp.tile([128, FREE], F32, tag="d_", name="d_")
                nc.vector.tensor_sub(d_, ht, h[p])
                zd = tmp.tile([128, FREE], F32, tag="zd", name="zd")
                nc.vector.tensor_mul(zd, z_sb, d_)
                nc.vector.tensor_add(h[q], h[p], zd)

                nc.sync.dma_start(out=out_d[ds(off, 128), :], in_=h[q])

                # u_next = x_{t+1} + h_new, refill x slot
                nc.vector.tensor_add(u[q], xs[s], h[q])
                if cast_rhs:
                    cast_mov(ub[q], u[q])
                nc.sync.dma_start(
                    out=xs[s], in_=x_d[ds(off + (unroll + 1) * 128, 128), :]
                )

            def substep_probe(s, off):
                """192 dependency-free MMs: measures pure PE MM throughput."""
                r_ps = psum.tile([128, FREE], F32, tag="r_ps", name="r_ps")
                z_ps = psum.tile([128, FREE], F32, tag="z_ps", name="z_ps")
                c_ps = psum.tile([128, FREE], F32, tag="c_ps", name="c_ps")
                for ps, wt in ((r_ps, wr), (z_ps, wz), (c_ps, w)):
                    for m in range(NT):
                        for k in range(NT):
                            nc.tensor.matmul(
                                ps[:, ts(m, BSH)],
                                wt[:, ds((m * NT + k) * 128, 128)],
                                ub[0][:, ts(k, BSH)],
                                start=(k == 0),
                                stop=(k == NT - 1),
                            )
                # drain psum so the pool recycles without a stall chain
                if s == unroll - 1:
                    hcp = tmp.tile([128, FREE], F32, tag="hcp", name="hcp")
                    nc.vector.tensor_copy(hcp, c_ps)
                    nc.sync.dma_start(out=out_d[ds(off, 128), :], in_=hcp)

            sstep = substep_probe if probe else substep

            def one_rep():
                prologue()
                with tc.For_i(
                    0, seq * 128, unroll * 128, staggered_reset=staggered
                ) as i0:
                    for s in range(unroll):
                        sstep(s, i0 + s * 128)

            if hwreps:
                # constant program size: repeat via an outer HARDWARE loop so
                # wall-clock deltas between reps counts measure pure HW exec
                with tc.For_i(0, reps, 1):
                    one_rep()
            else:
                for _rep in range(reps):
                    one_rep()

    nc.finalize()
    return nc


def _prep_weights(wg, dt):
    # stationary tile (m,k): lhsT[p, c] = scale * Wg[m*128+c, k*128+p]
    scale = W_SCALE[dt]
    return np.ascontiguousarray(
        (scale * wg).reshape(NT, 128, NT, 128).transpose(3, 0, 2, 1).reshape(128, -1)
    ).astype(NP_BY_DT[dt])


def _prep_x(x_shard):
    # x_shard [seq, BSH, DIM] -> [PAD*128, FREE]; [t*128+p, m*8+j] = x[t, j, m*128+p]
    seq = x_shard.shape[0]
    xp = np.zeros((PAD, 128, FREE), dtype=np.float32)
    xp[:seq] = (
        x_shard.reshape(seq, BSH, NT, 128).transpose(0, 3, 2, 1).reshape(seq, 128, FREE)
    )
    return xp.reshape(PAD * 128, FREE)


def prep_in_maps(x, Wz, Wr, W, b, cfg):
    wz_dt, wr_dt, w_dt = CONFIGS[cfg]
    wz_p = _prep_weights(Wz, wz_dt)
    wr_p = _prep_weights(Wr, wr_dt)
    w_p = _prep_weights(W, w_dt)
    # bias2[p, m*8+j] = Sw * 2*b[m*128+p]  (pre-scaled so tanh's 1/Sw undoes it)
    sw = W_SCALE[w_dt] * MOV_SCALE.get(cfg, 1.0)
    bias2 = np.ascontiguousarray(
        np.broadcast_to(
            (sw * 2.0 * b).reshape(NT, 128).T[:, :, None], (128, NT, BSH)
        ).reshape(128, FREE)
    ).astype(np.float32)

    in_maps = []
    for c in range(NCORES):
        xs = x[:, c * BSH : (c + 1) * BSH, :]
        in_maps.append(
            {
                "x": _prep_x(xs),
                "wz": wz_p,
                "wr": wr_p,
                "w": w_p,
                "bias2": bias2,
            }
        )
    return in_maps


def unpack_out(res, seq):
    outs = []
    for c in range(NCORES):
        o = np.asarray(res.results[c]["out"], dtype=np.float32)
        # [seq*128, FREE] -> [seq, BSH, DIM]
        o = (
            o.reshape(seq, 128, NT, BSH)
            .transpose(0, 3, 2, 1)
            .reshape(seq, BSH, DIM)
        )
        outs.append(o)
    return np.concatenate(outs, axis=1)


_CACHE = {}
LAST_RESULT = None


def kernel(x, Wz, Wr, W, b, unroll=8, w_dt_name="bfloat16", trace=False, reps=1):
    x = np.asarray(x, dtype=np.float32)
    Wz = np.asarray(Wz, dtype=np.float32)
    Wr = np.asarray(Wr, dtype=np.float32)
    W = np.asarray(W, dtype=np.float32)
    b = np.asarray(b, dtype=np.float32)
    seq = x.shape[0]

    key = (seq, unroll, w_dt_name, reps)
    if key not in _CACHE:
        _CACHE[key] = build_nc(seq=seq, unroll=unroll, cfg=w_dt_name, reps=reps)
    nc = _CACHE[key]

    in_maps = prep_in_maps(x, Wz, Wr, W, b, w_dt_name)

    global LAST_RESULT
    res = run_bass_kernel_spmd(nc, in_maps, list(range(NCORES)), trace=trace)
    LAST_RESULT = res
    return unpack_out(res, seq)
